# revision 1
# baseline (speedup 1.0000x reference)
"""BoneMeshGAT Trainium2 kernel: 3-layer GAT + BN/ELU on 8 NeuronCores.

Sharding: edges partitioned by dst across 8 cores (each core owns a
contiguous 12.5k-node dst shard and all edges into it). Per layer, each
core aggregates its shard's nodes (segment-softmax attention via
selection-matrix matmuls into PSUM), BN stats are AllReduced, the next
layer's gather table is rebuilt per-shard and AllGathered. Gathers of
per-edge source rows use indirect DMA. The program is JIT-built per
input graph (block/tile structure baked from the sorted edge list).
"""
import sys
sys.path.insert(0, "/opt/trn_rl_repo")
import numpy as np

import concourse.bass as bass
import concourse.mybir as mybir
import concourse.tile as tile
from concourse import bacc, bass_utils

N, E, G = 100000, 1000000, 16
H, D, IN = 4, 64, 4
HID = H * D
NC_ = 8
P = 128
NSH = N // NC_              # 12500 nodes per shard
NBLK = (NSH + P - 1) // P   # 98 blocks
NSHP = NBLK * P             # 12544 padded rows
EPS = 1e-5
F32 = mybir.dt.float32
I32 = mybir.dt.int32


def _fold_attn(W, a_src, a_dst, heads, d):
    # asrc_vec[i, h] = sum_d W[i, h*d+dd] * a_src[h, dd]
    Wr = W.reshape(W.shape[0], heads, d)
    av = np.einsum('ihd,hd->ih', Wr, a_src).astype(np.float32)
    bv = np.einsum('ihd,hd->ih', Wr, a_dst).astype(np.float32)
    return av, bv


def _host_prep(x, edge_index):
    src = edge_index[0].astype(np.int64)
    dst = edge_index[1].astype(np.int64)
    order = np.argsort(dst, kind='stable')
    s_s, d_s = src[order], dst[order]
    core_bounds = np.searchsorted(d_s, np.arange(0, N + 1, NSH))
    cnts = np.zeros((NC_, NBLK), np.int64)
    per_core = []
    for c in range(NC_):
        e0, e1 = core_bounds[c], core_bounds[c + 1]
        sc, dc = s_s[e0:e1], d_s[e0:e1] - c * NSH
        db = dc // P
        cnts[c] = np.bincount(db, minlength=NBLK)
        per_core.append((sc, dc, db))
    tpb = np.maximum(1, -(-cnts.max(axis=0) // P))  # tiles per block, shared
    TT = int(tpb.sum())
    tile_start = np.zeros(NBLK + 1, np.int64)
    tile_start[1:] = np.cumsum(tpb)

    src_slab = np.zeros((NC_, P, TT), np.int32)
    dcol_slab = np.full((NC_, P, TT), 999.0, np.float32)
    xsrcT_slab = np.zeros((NC_, IN, TT * P), np.float32)
    for c in range(NC_):
        sc, dc, db = per_core[c]
        blk_off = np.zeros(NBLK + 1, np.int64)
        blk_off[1:] = np.cumsum(cnts[c])
        for j in range(NBLK):
            e0, e1 = blk_off[j], blk_off[j + 1]
            n = e1 - e0
            t0 = tile_start[j]
            for k in range(int(tpb[j])):
                a = e0 + k * P
                b = min(a + P, e1)
                if b <= a:
                    break
                m = b - a
                col = t0 + k
                srcs = sc[a:b]
                # remap to padded AllGather row ids
                rows = (srcs // NSH) * NSHP + (srcs % NSH)
                src_slab[c, :m, col] = rows.astype(np.int32)
                dcol_slab[c, :m, col] = (dc[a:b] - j * P).astype(np.float32)
                xsrcT_slab[c, :, col * P:col * P + m] = x[srcs].T
    return tpb, TT, src_slab, dcol_slab, xsrcT_slab


def _build_program(TT, tpb, weights):
    (M1, W2f, W3f, g1, b1, g2, b2, g3, b3) = weights
    RW2, RW3 = 264, 66
    nc = bacc.Bacc("TRN2", target_bir_lowering=False, debug=False,
                   num_devices=NC_)

    def din(name, shape, dt=F32):
        return nc.dram_tensor(name, shape, dt, kind="ExternalInput").ap()

    i_src = din("i_src", [P, TT], I32)
    i_dcol = din("i_dcol", [P, TT])
    i_xsrcT = din("i_xsrcT", [IN, TT * P])
    i_xshT = din("i_xshT", [IN, NSHP])
    i_iota = din("i_iota", [P, P])
    i_ident = din("i_ident", [P, P])
    i_ones1 = din("i_ones1", [1, P])
    i_M1 = din("i_M1", [IN, RW2])
    i_W2f = din("i_W2f", [HID, RW2])
    i_W3f = din("i_W3f", [HID, RW3])
    i_g1 = din("i_g1", [1, HID]); i_b1 = din("i_b1", [1, HID])
    i_g2 = din("i_g2", [1, HID]); i_b2 = din("i_b2", [1, HID])
    i_g3 = din("i_g3", [1, D]);   i_b3 = din("i_b3", [1, D])
    o_h3 = nc.dram_tensor("o_h3", [NSHP, D], F32, kind="ExternalOutput").ap()

    with tile.TileContext(nc) as tc:
        with (
            tc.tile_pool(name="dram", bufs=1, space="DRAM") as dp,
            tc.tile_pool(name="const", bufs=1) as cp,
        ):
            agg1 = dp.tile([NSHP, HID], F32)
            agg2 = dp.tile([NSHP, HID], F32)
            agg3 = dp.tile([NSHP, D], F32)
            t2_own = dp.tile([NSHP, RW2], F32)
            t2_full = dp.tile([NC_ * NSHP, RW2], F32, addr_space="Shared")
            t3_own = dp.tile([NSHP, RW3], F32)
            t3_full = dp.tile([NC_ * NSHP, RW3], F32, addr_space="Shared")
            st1_in = dp.tile([2, HID], F32)
            st1_out = dp.tile([2, HID], F32, addr_space="Shared")
            st2_in = dp.tile([2, HID], F32)
            st2_out = dp.tile([2, HID], F32, addr_space="Shared")
            st3_in = dp.tile([2, D], F32)
            st3_out = dp.tile([2, D], F32, addr_space="Shared")

            iota = cp.tile([P, P], F32)
            nc.sync.dma_start(iota[:], i_iota[:])
            ident = cp.tile([P, P], F32)
            nc.sync.dma_start(ident[:], i_ident[:])
            ones1 = cp.tile([1, P], F32)
            nc.sync.dma_start(ones1[:], i_ones1[:])
            onesc = cp.tile([P, 1], F32)
            nc.vector.memset(onesc[:], 1.0)
            src_slab = cp.tile([P, TT], I32)
            nc.sync.dma_start(src_slab[:], i_src[:])
            dcol_slab = cp.tile([P, TT], F32)
            nc.sync.dma_start(dcol_slab[:], i_dcol[:])
            M1sb = cp.tile([IN, RW2], F32)
            nc.sync.dma_start(M1sb[:], i_M1[:])

            # ---- L1 prep: s_dst1 per block from x_shardT @ M1[:,260:264]
            sdst1 = cp.tile([P, 4 * NBLK], F32)
            with tc.tile_pool(name="pp0", bufs=2, space="PSUM") as pp0, \
                 tc.tile_pool(name="sp0", bufs=2) as sp0:
                xsh = sp0.tile([IN, NSHP], F32, tag="xsh")
                nc.sync.dma_start(xsh[:], i_xshT[:])
                for j in range(NBLK):
                    ps = pp0.tile([P, 4], F32, tag="ps")
                    nc.tensor.matmul(out=ps[:], lhsT=xsh[:, j * P:(j + 1) * P],
                                     rhs=M1sb[:, 260:264], start=True, stop=True)
                    nc.vector.tensor_copy(sdst1[:, 4 * j:4 * j + 4], ps[:])

            CHT = 16  # xsrcT tiles per SBUF chunk (L1)

            def edge_phase(layer, table_ap, agg_ap, sdst_own_ap, sdst_cols,
                           xsrcT_ap, Hn, RW, numw, st_in_ap):
                """One GAT layer edge aggregation into agg_ap [NSHP, numw],
                with BN-stat partial sums fused in (written to st_in_ap)."""
                rhw = numw + Hn   # rhs/acc width
                ps_bufs = 1 if layer == 1 else 2
                with (
                    tc.tile_pool(name=f"g{layer}", bufs=8) as gp,
                    tc.tile_pool(name=f"w{layer}", bufs=4) as wp,
                    tc.tile_pool(name=f"b{layer}", bufs=3) as bp,
                    tc.tile_pool(name=f"x{layer}", bufs=2) as xp,
                    tc.tile_pool(name=f"ps{layer}", bufs=ps_bufs, space="PSUM") as pp,
                    tc.tile_pool(name=f"pa{layer}", bufs=2, space="PSUM") as pa,
                    tc.tile_pool(name=f"pq{layer}", bufs=1, space="PSUM") as pq,
                ):
                    s0 = pq.tile([1, numw], F32, tag="s0")
                    s1 = pq.tile([1, numw], F32, tag="s1")
                    xchunk = None
                    t = 0
                    for j in range(NBLK):
                        if layer == 1:
                            sdb = sdst1
                            sdb_sl = (slice(None), slice(4 * j, 4 * j + Hn))
                        else:
                            sdb = bp.tile([P, Hn], F32, tag="sdb")
                            nc.sync.dma_start(
                                sdb[:], sdst_own_ap[j * P:(j + 1) * P,
                                                    sdst_cols[0]:sdst_cols[1]])
                            sdb_sl = (slice(None), slice(None))
                        acc = pa.tile([P, rhw], F32, tag="acc")
                        ntile = int(tpb[j])
                        for k in range(ntile):
                            if layer == 1:
                                if t % CHT == 0:
                                    xchunk = xp.tile([IN, CHT * P], F32,
                                                     tag="xchunk")
                                    ncols = min(CHT * P, TT * P - t * P)
                                    nc.sync.dma_start(
                                        xchunk[:, :ncols],
                                        xsrcT_ap[:, t * P:t * P + ncols])
                                co = (t % CHT) * P
                                gps = pp.tile([P, RW], F32, tag="gps")
                                nc.tensor.matmul(
                                    out=gps[:],
                                    lhsT=xchunk[:, co:co + P],
                                    rhs=M1sb[:], start=True, stop=True)
                                g = wp.tile([P, RW], F32, tag="gsb")
                                nc.vector.tensor_copy(g[:], gps[:])
                            else:
                                g = gp.tile([P, RW], F32, tag="gsb")
                                nc.gpsimd.indirect_dma_start(
                                    out=g[:], out_offset=None, in_=table_ap,
                                    in_offset=bass.IndirectOffsetOnAxis(
                                        ap=src_slab[:, t:t + 1], axis=0))
                            sel = wp.tile([P, P], F32, tag="sel")
                            nc.vector.tensor_tensor(
                                out=sel[:],
                                in0=dcol_slab[:, t:t + 1].to_broadcast([P, P]),
                                in1=iota[:], op=mybir.AluOpType.is_equal)
                            selT_ps = pp.tile([P, P], F32, tag="selTps")
                            nc.tensor.transpose(out=selT_ps[:], in_=sel[:],
                                                identity=ident[:])
                            selT = wp.tile([P, P], F32, tag="selT")
                            nc.vector.tensor_copy(selT[:], selT_ps[:])
                            sd_ps = pp.tile([P, Hn], F32, tag="sdps")
                            nc.tensor.matmul(out=sd_ps[:], lhsT=selT[:],
                                             rhs=sdb[sdb_sl],
                                             start=True, stop=True)
                            lg = wp.tile([P, Hn], F32, tag="lg")
                            nc.vector.tensor_add(lg[:], g[:, numw:numw + Hn],
                                                 sd_ps[:])
                            lk = wp.tile([P, Hn], F32, tag="lk")
                            nc.vector.scalar_tensor_tensor(
                                out=lk[:], in0=lg[:], scalar=0.2, in1=lg[:],
                                op0=mybir.AluOpType.mult,
                                op1=mybir.AluOpType.max)
                            rhs = wp.tile([P, rhw], F32, tag="rhs")
                            nc.scalar.activation(
                                rhs[:, numw:numw + Hn], lk[:],
                                mybir.ActivationFunctionType.Exp)
                            for h in range(Hn):
                                nc.vector.tensor_scalar(
                                    out=rhs[:, h * D:(h + 1) * D],
                                    in0=g[:, h * D:(h + 1) * D],
                                    scalar1=rhs[:, numw + h:numw + h + 1],
                                    scalar2=None,
                                    op0=mybir.AluOpType.mult)
                            nc.tensor.matmul(out=acc[:], lhsT=sel[:], rhs=rhs[:],
                                             start=(k == 0), stop=(k == ntile - 1))
                            t += 1
                        # softmax division: num/(den+1e-8)
                        dp_ = bp.tile([P, Hn], F32, tag="dp")
                        nc.vector.tensor_scalar_add(dp_[:], acc[:, numw:numw + Hn],
                                                    1e-8)
                        rec = bp.tile([P, Hn], F32, tag="rec")
                        nc.vector.reciprocal(rec[:], dp_[:])
                        aggb = bp.tile([P, numw], F32, tag="aggb")
                        for h in range(Hn):
                            nc.vector.tensor_scalar(
                                out=aggb[:, h * D:(h + 1) * D],
                                in0=acc[:, h * D:(h + 1) * D],
                                scalar1=rec[:, h:h + 1], scalar2=None,
                                op0=mybir.AluOpType.mult)
                        nc.sync.dma_start(agg_ap[j * P:(j + 1) * P, :], aggb[:])
                        sq = bp.tile([P, numw], F32, tag="sq")
                        nc.vector.tensor_mul(sq[:], aggb[:], aggb[:])
                        nc.tensor.matmul(out=s0[:], lhsT=onesc[:], rhs=aggb[:],
                                         start=(j == 0), stop=(j == NBLK - 1))
                        nc.tensor.matmul(out=s1[:], lhsT=onesc[:], rhs=sq[:],
                                         start=(j == 0), stop=(j == NBLK - 1))
                    assert t == TT
                    c0 = bp.tile([1, numw], F32, tag="c0")
                    nc.vector.tensor_copy(c0[:], s0[:])
                    c1 = bp.tile([1, numw], F32, tag="c1")
                    nc.vector.tensor_copy(c1[:], s1[:])
                    nc.sync.dma_start(st_in_ap[0:1, :], c0[:])
                    nc.sync.dma_start(st_in_ap[1:2, :], c1[:])

            def stats_ar(st_in_ap, st_out_ap):
                nc.gpsimd.collective_compute(
                    "AllReduce", mybir.AluOpType.add,
                    ins=[st_in_ap], outs=[st_out_ap],
                    replica_groups=[list(range(NC_))])

            def bn_params(st_out_ap, CH, g_ap, b_ap, pool, tag=""):
                """Returns replicated A,B tiles [P, CH]: x_hat*g+b = x*A+B."""
                st0 = pool.tile([1, CH], F32, tag="stl0")
                nc.sync.dma_start(st0[:], st_out_ap[0:1, :])
                st1_ = pool.tile([1, CH], F32, tag="stl1")
                nc.sync.dma_start(st1_[:], st_out_ap[1:2, :])
                grow = pool.tile([1, CH], F32, tag="grow")
                nc.sync.dma_start(grow[:], g_ap[:])
                brow = pool.tile([1, CH], F32, tag="brow")
                nc.sync.dma_start(brow[:], b_ap[:])
                mu = pool.tile([1, CH], F32, tag="mu")
                nc.vector.tensor_scalar_mul(mu[:], st0[:], 1.0 / N)
                msq = pool.tile([1, CH], F32, tag="msq")
                nc.vector.tensor_scalar_mul(msq[:], st1_[:], 1.0 / N)
                var = pool.tile([1, CH], F32, tag="var")
                nc.vector.tensor_mul(var[:], mu[:], mu[:])
                nc.vector.tensor_sub(var[:], msq[:], var[:])
                nc.vector.tensor_scalar_add(var[:], var[:], EPS)
                sd = pool.tile([1, CH], F32, tag="sd")
                nc.scalar.activation(sd[:], var[:],
                                     mybir.ActivationFunctionType.Sqrt)
                r = pool.tile([1, CH], F32, tag="r")
                nc.vector.reciprocal(r[:], sd[:])
                arow = pool.tile([1, CH], F32, tag="arow")
                nc.vector.tensor_mul(arow[:], r[:], grow[:])
                brow2 = pool.tile([1, CH], F32, tag="brow2")
                nc.vector.tensor_mul(brow2[:], mu[:], arow[:])
                nc.vector.tensor_sub(brow2[:], brow[:], brow2[:])
                with tc.tile_pool(name=f"pbn{tag}", bufs=1, space="PSUM") as pb:
                    aps = pb.tile([P, CH], F32, tag="aps")
                    nc.tensor.matmul(out=aps[:], lhsT=ones1[:], rhs=arow[:],
                                     start=True, stop=True)
                    A = pool.tile([P, CH], F32, tag="Arep")
                    nc.vector.tensor_copy(A[:], aps[:])
                    bps = pb.tile([P, CH], F32, tag="bps")
                    nc.tensor.matmul(out=bps[:], lhsT=ones1[:], rhs=brow2[:],
                                     start=True, stop=True)
                    B = pool.tile([P, CH], F32, tag="Brep")
                    nc.vector.tensor_copy(B[:], bps[:])
                return A, B

            def bn_elu_block(xb, A, B, CH, pool):
                """BN apply + ELU in place; returns act tile [P, CH]."""
                z = pool.tile([P, CH], F32, tag="z")
                nc.vector.tensor_mul(z[:], xb[:], A[:])
                nc.vector.tensor_add(z[:], z[:], B[:])
                m0 = pool.tile([P, CH], F32, tag="m0")
                nc.vector.tensor_scalar_min(m0[:], z[:], 0.0)
                e = pool.tile([P, CH], F32, tag="e")
                nc.scalar.activation(e[:], m0[:],
                                     mybir.ActivationFunctionType.Exp)
                p0 = pool.tile([P, CH], F32, tag="p0")
                nc.vector.tensor_scalar_max(p0[:], z[:], 0.0)
                act = pool.tile([P, CH], F32, tag="act")
                nc.vector.tensor_add(act[:], p0[:], e[:])
                nc.vector.tensor_scalar_sub(act[:], act[:], 1.0)
                return act

            def build_pass(agg_ap, st_out_ap, g_ap, b_ap, Wf_ap, RW,
                           t_own_ap, t_full_ap, tag=""):
                CH = HID
                with (
                    tc.tile_pool(name=f"bps{tag}", bufs=3) as sp,
                    tc.tile_pool(name=f"bpc{tag}", bufs=1) as cpool,
                    tc.tile_pool(name=f"bpp{tag}", bufs=2, space="PSUM") as pp,
                    tc.tile_pool(name=f"bpt{tag}", bufs=2, space="PSUM") as pt,
                ):
                    A, B = bn_params(st_out_ap, CH, g_ap, b_ap, cpool, tag)
                    Wsb0 = cpool.tile([P, RW], F32, tag="Wsb0")
                    nc.sync.dma_start(Wsb0[:], Wf_ap[0:P, :])
                    Wsb1 = cpool.tile([P, RW], F32, tag="Wsb1")
                    nc.sync.dma_start(Wsb1[:], Wf_ap[P:2 * P, :])
                    Wsb = [Wsb0, Wsb1]
                    for j in range(NBLK):
                        xb = sp.tile([P, CH], F32, tag="xb")
                        nc.sync.dma_start(xb[:], agg_ap[j * P:(j + 1) * P, :])
                        act = bn_elu_block(xb, A, B, CH, sp)
                        tp = pt.tile([P, RW], F32, tag="tp")
                        for k in range(2):
                            trp = pp.tile([P, P], F32, tag="trp")
                            nc.tensor.transpose(
                                out=trp[:], in_=act[:, k * P:(k + 1) * P],
                                identity=ident[:])
                            atk = sp.tile([P, P], F32, tag="atk")
                            nc.vector.tensor_copy(atk[:], trp[:])
                            nc.tensor.matmul(out=tp[:], lhsT=atk[:],
                                             rhs=Wsb[k][:],
                                             start=(k == 0), stop=(k == 1))
                        tt = sp.tile([P, RW], F32, tag="tt")
                        nc.vector.tensor_copy(tt[:], tp[:])
                        nc.sync.dma_start(t_own_ap[j * P:(j + 1) * P, :], tt[:])
                nc.gpsimd.collective_compute(
                    "AllGather", mybir.AluOpType.bypass,
                    ins=[t_own_ap], outs=[t_full_ap],
                    replica_groups=[list(range(NC_))])

            # ================= pipeline =================
            edge_phase(1, None, agg1[:], None, None,
                       i_xsrcT[:], H, RW2, HID, st1_in[:])
            stats_ar(st1_in[:], st1_out[:])
            build_pass(agg1[:], st1_out[:], i_g1[:], i_b1[:], i_W2f[:], RW2,
                       t2_own[:], t2_full[:], tag="1")
            edge_phase(2, t2_full[:], agg2[:], t2_own[:], (260, 264),
                       None, H, RW2, HID, st2_in[:])
            stats_ar(st2_in[:], st2_out[:])
            build_pass(agg2[:], st2_out[:], i_g2[:], i_b2[:], i_W3f[:], RW3,
                       t3_own[:], t3_full[:], tag="2")
            edge_phase(3, t3_full[:], agg3[:], t3_own[:], (65, 66),
                       None, 1, RW3, D, st3_in[:])
            stats_ar(st3_in[:], st3_out[:])
            # final BN3 + ELU -> h3 output
            with (
                tc.tile_pool(name="f3", bufs=3) as sp,
                tc.tile_pool(name="f3c", bufs=1) as cpool,
            ):
                A3, B3 = bn_params(st3_out[:], D, i_g3[:], i_b3[:], cpool, "3")
                for j in range(NBLK):
                    xb = sp.tile([P, D], F32, tag="xb")
                    nc.sync.dma_start(xb[:], agg3[j * P:(j + 1) * P, :])
                    act = bn_elu_block(xb, A3, B3, D, sp)
                    nc.sync.dma_start(o_h3[j * P:(j + 1) * P, :], act[:])
    nc.compile()
    return nc


def kernel(**inputs):
    x = np.asarray(inputs["x"], np.float32)
    edge_index = np.asarray(inputs["edge_index"], np.int32)
    batch = np.asarray(inputs["batch"], np.int64)
    W1 = np.asarray(inputs["W1"], np.float32)
    a_src1 = np.asarray(inputs["a_src1"], np.float32)
    a_dst1 = np.asarray(inputs["a_dst1"], np.float32)
    g1 = np.asarray(inputs["g1"], np.float32); b1 = np.asarray(inputs["b1"], np.float32)
    W2 = np.asarray(inputs["W2"], np.float32)
    a_src2 = np.asarray(inputs["a_src2"], np.float32)
    a_dst2 = np.asarray(inputs["a_dst2"], np.float32)
    g2 = np.asarray(inputs["g2"], np.float32); b2 = np.asarray(inputs["b2"], np.float32)
    W3 = np.asarray(inputs["W3"], np.float32)
    a_src3 = np.asarray(inputs["a_src3"], np.float32)
    a_dst3 = np.asarray(inputs["a_dst3"], np.float32)
    g3 = np.asarray(inputs["g3"], np.float32); b3 = np.asarray(inputs["b3"], np.float32)
    Wc1 = np.asarray(inputs["Wc1"], np.float32); bc1 = np.asarray(inputs["bc1"], np.float32)
    gc1 = np.asarray(inputs["gc1"], np.float32); bnc1 = np.asarray(inputs["bnc1"], np.float32)
    Wc2 = np.asarray(inputs["Wc2"], np.float32); bc2 = np.asarray(inputs["bc2"], np.float32)
    Wc3 = np.asarray(inputs["Wc3"], np.float32); bc3 = np.asarray(inputs["bc3"], np.float32)

    tpb, TT, src_slab, dcol_slab, xsrcT_slab = _host_prep(x, edge_index)

    av1, bv1 = _fold_attn(W1, a_src1, a_dst1, H, D)
    M1 = np.concatenate([W1, av1, bv1], axis=1).astype(np.float32)      # [4,264]
    av2, bv2 = _fold_attn(W2, a_src2, a_dst2, H, D)
    W2f = np.concatenate([W2, av2, bv2], axis=1).astype(np.float32)     # [256,264]
    av3 = (W3 @ a_src3[0])[:, None].astype(np.float32)
    bv3 = (W3 @ a_dst3[0])[:, None].astype(np.float32)
    W3f = np.concatenate([W3, av3, bv3], axis=1).astype(np.float32)     # [256,66]

    nc = _build_program(TT, tpb, (M1, W2f, W3f, g1, b1, g2, b2, g3, b3))

    xshT = np.zeros((NC_, IN, NSHP), np.float32)
    for c in range(NC_):
        xshT[c, :, :NSH] = x[c * NSH:(c + 1) * NSH].T
    consts = {
        "i_iota": np.tile(np.arange(P, dtype=np.float32)[None, :], (P, 1)),
        "i_ident": np.eye(P, dtype=np.float32),
        "i_ones1": np.ones((1, P), np.float32),
        "i_M1": M1, "i_W2f": W2f, "i_W3f": W3f,
        "i_g1": g1[None, :], "i_b1": b1[None, :],
        "i_g2": g2[None, :], "i_b2": b2[None, :],
        "i_g3": g3[None, :], "i_b3": b3[None, :],
    }
    in_maps = []
    for c in range(NC_):
        in_maps.append({
            "i_src": src_slab[c], "i_dcol": dcol_slab[c],
            "i_xsrcT": xsrcT_slab[c], "i_xshT": xshT[c], **consts})

    res = bass_utils.run_bass_kernel_spmd(nc, in_maps,
                                          core_ids=list(range(NC_)))
    h3 = np.concatenate(
        [res.results[c]["o_h3"][:NSH] for c in range(NC_)], axis=0)

    # ---- host: graph pooling + classifier MLP (fp32 mirror of reference)
    counts = np.bincount(batch, minlength=G).astype(np.float32)
    h_sum = np.zeros((G, D), np.float32)
    np.add.at(h_sum, batch, h3)
    h_mean = h_sum / (counts[:, None] + 1.0)
    h_max = np.full((G, D), -np.inf, np.float32)
    np.maximum.at(h_max, batch, h3)
    h_max = np.maximum(h_max, 0.0)
    gfeat = np.concatenate([h_mean, h_max], axis=1)
    z = gfeat @ Wc1 + bc1
    mu = z.mean(0); var = z.var(0)
    z = (z - mu) / np.sqrt(var + EPS) * gc1 + bnc1
    z = np.maximum(z, 0.0)
    z = np.maximum(z @ Wc2 + bc2, 0.0)
    return (z @ Wc3 + bc3).astype(np.float32)



# revision 27
# speedup vs baseline: 1.0116x; 1.0116x over previous
"""BoneMeshGAT Trainium2 kernel: 3-layer GAT + BN/ELU on 8 NeuronCores.

Edges partitioned by dst across 8 cores (contiguous 12.5k-node dst shard
per core). fp16 edge pipeline:
 - per-node tables (rows [h | a_src | pad], 384/128 elems) AllGathered,
   fetched per-edge with InstDMAGatherAnt (int16 idx, table split in
   QSIZE-row sub-ranges; tiles quarter-sorted so each (group, quarter)
   is one gather)
 - dst attention term from an own-shard adst table via selection-matrix
   matmuls (sel/selT 0/1 slabs prebuilt on host, streamed from DRAM)
 - segment softmax + scatter-add via sel^T matmuls, PSUM fp32 accum
 - features stored head-interleaved (pos d*H+h) so the big per-edge
   multiply runs in DVE 2x mode; weights/BN params permuted on host
 - layer-1 logits folded on host into the per-edge slab; layer-1
   aggregates 4-wide x then expands through a block-diagonal W1
 - BN stats fused into edge phases (ones-vector matmuls), AllReduced.
"""
import sys
sys.path.insert(0, "/opt/trn_rl_repo")
import numpy as np

import concourse.bass as bass
import concourse.mybir as mybir
import concourse.tile as tile
from concourse import bacc, bass_utils
from concourse.ap import AP

N, E, G = 100000, 1000000, 16
H, D, IN = 4, 64, 4
HID = H * D
NC_ = 8
P = 128
EPS = 1e-5
F16 = mybir.dt.float16
F32 = mybir.dt.float32
I32 = mybir.dt.int32
I16 = mybir.dt.int16
ALU = mybir.AluOpType
ACT = mybir.ActivationFunctionType

QSIZE = 25088       # rows per gather sub-table (int16 idx limit)
GB = 4              # blocks per gather group
W2ROW = 384         # padded t2 row elems (768B, %256==0)
W3ROW = 128         # padded t3 row elems (256B)


def _derived(n):
    nsh = n // NC_
    nblk = (nsh + P - 1) // P
    return nsh, nblk, nblk * P


def _perm():
    """Head-interleave permutation: new[d*H+h] = old[h*D+d]."""
    p = np.zeros(HID, np.int64)
    for h in range(H):
        for d in range(D):
            p[d * H + h] = h * D + d
    return p


def _fold_attn(W, a_src, a_dst, heads, d):
    Wr = W.reshape(W.shape[0], heads, d)
    av = np.einsum('ihd,hd->ih', Wr, a_src).astype(np.float32)
    bv = np.einsum('ihd,hd->ih', Wr, a_dst).astype(np.float32)
    return av, bv


class Layout:
    """Slab/tile layout shared across cores (program structure)."""
    def __init__(self, tpb_q, nblk, nq):
        self.nblk, self.nq = nblk, nq
        self.groups = []   # dicts: j0, j1, ts, te, segs, runs
        self.rs = np.zeros((nblk, nq), np.int64)   # first slab tile of (j,q)
        pos = 0
        for j0 in range(0, nblk, GB):
            j1 = min(j0 + GB, nblk)
            gd = {"j0": j0, "j1": j1, "ts": pos, "segs": [], "runs": {}}
            for q in range(nq):
                ts = pos
                for j in range(j0, j1):
                    self.rs[j, q] = pos
                    k = int(tpb_q[j, q])
                    if k:
                        gd["runs"].setdefault(j, []).append(
                            (q, pos, pos + k))
                    pos += k
                if pos > ts:
                    gd["segs"].append((q, ts, pos))
            gd["te"] = pos
            self.groups.append(gd)
        self.TT = pos
        self.ktot = tpb_q.sum(axis=1).astype(np.int64)  # tiles per block


def _host_prep(x, edge_index, W1, a_src1, a_dst1):
    nsh, nblk, nshp = _derived(N)
    rows_total = NC_ * nshp
    nq = -(-rows_total // QSIZE)
    src = edge_index[0].astype(np.int64)
    dst = edge_index[1].astype(np.int64)
    order = np.argsort(dst, kind='stable')
    s_s, d_s = src[order], dst[order]
    core_bounds = np.searchsorted(d_s, np.arange(0, N + 1, nsh))

    av1, bv1 = _fold_attn(W1, a_src1, a_dst1, H, D)  # [IN, H]

    per_core = []
    cnt = np.zeros((NC_, nblk, nq), np.int64)
    for c in range(NC_):
        e0, e1 = core_bounds[c], core_bounds[c + 1]
        sc, dc = s_s[e0:e1], d_s[e0:e1] - c * nsh
        row = (sc // nsh) * nshp + (sc % nsh)
        q = row // QSIZE
        j = dc // P
        o2 = np.argsort(j * nq + q, kind='stable')
        sc, dc, row, q, j = sc[o2], dc[o2], row[o2], q[o2], j[o2]
        np.add.at(cnt[c], (j, q), 1)
        per_core.append((sc, dc, row, q, j))

    tpb_q = -(-cnt.max(axis=0) // P)          # [nblk, nq]
    for j in range(nblk):                     # ensure >=1 tile per block
        if tpb_q[j].sum() == 0:
            tpb_q[j, 0] = 1
    lay = Layout(tpb_q, nblk, nq)
    TT = lay.TT

    xe_slab = np.zeros((NC_, P, TT * 2 * IN), np.float16)
    selT_slab = np.zeros((NC_, P, TT * P), np.float16)
    dcol_slab = np.full((NC_, P, TT), 999.0, np.float32)
    islab = np.zeros((NC_, P, TT * 8), np.int16)

    g_of_j = np.zeros(nblk, np.int64)
    for gi, gd in enumerate(lay.groups):
        g_of_j[gd["j0"]:gd["j1"]] = gi

    for c in range(NC_):
        sc, dc, row, qe, je = per_core[c]
        xs = x[sc]
        lg1 = (xs @ av1 + x[dc + c * nsh] @ bv1).astype(np.float32)
        flat = np.zeros(nblk * nq + 1, np.int64)
        flat[1:] = np.cumsum(cnt[c].reshape(-1))
        for j in range(nblk):
            for q in range(nq):
                a = flat[j * nq + q]
                b = flat[j * nq + q + 1]
                m = b - a
                if m == 0:
                    continue
                rs = lay.rs[j, q]
                k_e = np.arange(m) // P
                p_e = np.arange(m) % P
                col = rs + k_e
                dcol = (dc[a:b] - j * P).astype(np.int64)
                xe_full = np.concatenate(
                    [xs[a:b], lg1[a:b]], axis=1).astype(np.float16)
                for i8 in range(2 * IN):
                    xe_slab[c, p_e, col * 2 * IN + i8] = xe_full[:, i8]
                selT_slab[c, dcol, col * P + p_e] = 1.0
                dcol_slab[c, p_e, col] = dcol
                ts = lay.groups[g_of_j[j]]["segs"]
                ts_gq = next(t for (qq, t, _) in ts if qq == q)
                s_e = (rs - ts_gq + k_e) * P + p_e
                islab[c, s_e % 16, ts_gq * 8 + s_e // 16] = (
                    row[a:b] - q * QSIZE).astype(np.int16)
        islab[c, 16:, :] = np.tile(islab[c, :16, :], (7, 1))
    return lay, xe_slab, selT_slab, dcol_slab, islab


def _ap3(t, off, dims):
    a = t[:] if not isinstance(t, AP) else t
    return AP(a.tensor, a.offset + off, [list(a.ap[0])] + [list(d) for d in dims])


def _build_program(lay, nblk, nshp):
    TT = lay.TT
    ROWS = NC_ * nshp
    nq = lay.nq
    nc = bacc.Bacc("TRN2", target_bir_lowering=False, debug=False,
                   num_devices=NC_)

    def din(name, shape, dt=F32):
        return nc.dram_tensor(name, shape, dt, kind="ExternalInput").ap()

    i_xe = din("i_xe", [P, TT * 2 * IN], F16)
    i_selT = din("i_selT", [P, TT * P], F16)
    i_dcol = din("i_dcol", [P, TT])
    i_iota = din("i_iota", [P, P], F16)
    i_islab = din("i_islab", [P, TT * 8], I16)
    i_ident = din("i_ident", [P, P], F16)
    i_ones1 = din("i_ones1", [1, P], F16)
    i_W1 = din("i_W1", [IN * H, HID], F16)
    i_W2f = din("i_W2f", [HID, HID + 2 * H], F16)
    i_W3f = din("i_W3f", [HID, D + 2], F16)
    i_g1 = din("i_g1", [1, HID]); i_b1 = din("i_b1", [1, HID])
    i_g2 = din("i_g2", [1, HID]); i_b2 = din("i_b2", [1, HID])
    i_g3 = din("i_g3", [1, D]);   i_b3 = din("i_b3", [1, D])
    o_h3 = nc.dram_tensor("o_h3", [nshp, D], F32, kind="ExternalOutput").ap()

    with tile.TileContext(nc) as tc:
        with (
            tc.tile_pool(name="dram", bufs=1, space="DRAM") as dp,
            tc.tile_pool(name="const", bufs=1) as cp,
        ):
            agg1 = dp.tile([nshp, HID], F16)
            agg2 = dp.tile([nshp, HID], F16)
            agg3 = dp.tile([nshp, D], F16)
            t2_own = dp.tile([nshp, W2ROW], F16)
            t2_full = dp.tile([ROWS, W2ROW], F16, addr_space="Shared")
            t3_own = dp.tile([nshp, W3ROW], F16)
            t3_full = dp.tile([ROWS, W3ROW], F16, addr_space="Shared")
            ad2_own = dp.tile([nshp, H], F16)
            ad3_own = dp.tile([nshp, 1], F16)
            st1_in = dp.tile([2, HID], F32)
            st1_out = dp.tile([2, HID], F32, addr_space="Shared")
            st2_in = dp.tile([2, HID], F32)
            st2_out = dp.tile([2, HID], F32, addr_space="Shared")
            st3_in = dp.tile([2, D], F32)
            st3_out = dp.tile([2, D], F32, addr_space="Shared")

            ident = cp.tile([P, P], F16)
            nc.sync.dma_start(ident[:], i_ident[:])
            iota = cp.tile([P, P], F16)
            nc.sync.dma_start(iota[:], i_iota[:])
            dcol = cp.tile([P, TT], F32)
            nc.sync.dma_start(dcol[:], i_dcol[:])
            ones1 = cp.tile([1, P], F16)
            nc.sync.dma_start(ones1[:], i_ones1[:])
            onesc = cp.tile([P, 1], F16)
            nc.vector.memset(onesc[:], 1.0)
            W1sb = cp.tile([IN * H, HID], F16)
            nc.sync.dma_start(W1sb[:], i_W1[:])
            islab = cp.tile([P, TT * 8], I16)
            nc.sync.dma_start(islab[:], i_islab[:])

            def edge1():
                with (
                    tc.tile_pool(name="e1g", bufs=2) as gp,
                    tc.tile_pool(name="e1w", bufs=2) as wp,
                    tc.tile_pool(name="e1b", bufs=3) as bp,
                    tc.tile_pool(name="e1pa", bufs=2, space="PSUM") as pa,
                    tc.tile_pool(name="e1pt", bufs=2, space="PSUM") as pt,
                    tc.tile_pool(name="e1pp", bufs=2, space="PSUM") as pp,
                    tc.tile_pool(name="e1pq", bufs=1, space="PSUM") as pq,
                ):
                    s0 = pq.tile([1, HID], F32, tag="s0")
                    s1 = pq.tile([1, HID], F32, tag="s1")
                    RH = IN * H + H  # 20
                    for gd in lay.groups:
                        ts_g, te_g = gd["ts"], gd["te"]
                        ntg = te_g - ts_g
                        xe = gp.tile([P, ntg * 2 * IN], F16, tag="xe")
                        nc.sync.dma_start(
                            xe[:], i_xe[:, ts_g * 2 * IN:te_g * 2 * IN])
                        for j in range(gd["j0"], gd["j1"]):
                            K = int(lay.ktot[j])
                            selc = wp.tile([P, K * P], F16, tag="selc")
                            kb = 0
                            for (q, rs, re) in gd["runs"].get(j, []):
                                for t in range(rs, re):
                                    nc.vector.tensor_scalar(
                                        out=selc[:, (kb + t - rs) * P:
                                                 (kb + t - rs + 1) * P],
                                        in0=iota[:],
                                        scalar1=dcol[:, t:t + 1],
                                        scalar2=None, op0=ALU.is_equal)
                                kb += re - rs
                            ex = wp.tile([P, K * H], F16, tag="ex")
                            rhs = wp.tile([P, K * RH], F16, tag="rhs")
                            kb = 0
                            for (q, rs, re) in gd["runs"].get(j, []):
                                nr = re - rs
                                l0 = (rs - ts_g)
                                lk = wp.tile([P, nr * H], F16, tag="lk")
                                lg_ap = _ap3(xe, l0 * 2 * IN + IN,
                                             [[2 * IN, nr], [1, H]])
                                nc.vector.scalar_tensor_tensor(
                                    out=lk[:], in0=lg_ap, scalar=0.2,
                                    in1=lg_ap, op0=ALU.mult, op1=ALU.max)
                                nc.scalar.activation(
                                    ex[:, kb * H:(kb + nr) * H], lk[:],
                                    ACT.Exp)
                                nc.vector.tensor_tensor(
                                    out=_ap3(rhs, kb * RH,
                                             [[RH, nr], [1, IN * H]]),
                                    in0=_ap3(xe, l0 * 2 * IN,
                                             [[2 * IN, nr], [0, H], [1, IN]]),
                                    in1=_ap3(ex, kb * H,
                                             [[H, nr], [1, H], [0, IN]]),
                                    op=ALU.mult)
                                nc.vector.tensor_copy(
                                    _ap3(rhs, kb * RH + IN * H,
                                         [[RH, nr], [1, H]]),
                                    ex[:, kb * H:(kb + nr) * H])
                                kb += nr
                            acc = pa.tile([P, RH], F32, tag="acc")
                            for kb in range(K):
                                nc.tensor.matmul(
                                    out=acc[:],
                                    lhsT=selc[:, kb * P:(kb + 1) * P],
                                    rhs=rhs[:, kb * RH:(kb + 1) * RH],
                                    start=(kb == 0), stop=(kb == K - 1))
                            den = bp.tile([P, H], F32, tag="den")
                            nc.vector.tensor_scalar_add(
                                den[:], acc[:, IN * H:IN * H + H], 1e-8)
                            rec = bp.tile([P, H], F32, tag="rec")
                            nc.vector.reciprocal(rec[:], den[:])
                            rec16 = bp.tile([P, H], F16, tag="rec16")
                            nc.vector.tensor_scalar(
                                out=rec16[:], in0=rec[:], scalar1=60000.0,
                                scalar2=None, op0=ALU.min)
                            accn = bp.tile([P, IN * H], F16, tag="accn")
                            nc.vector.tensor_tensor(
                                out=accn[:], in0=acc[:, 0:IN * H],
                                in1=_ap3(rec16, 0, [[1, H], [0, IN]]),
                                op=ALU.mult)
                            trp = pt.tile([IN * H, P], F16, tag="trp")
                            nc.tensor.transpose(out=trp[:], in_=accn[:],
                                                identity=ident[:])
                            accT = bp.tile([IN * H, P], F16, tag="accT")
                            nc.scalar.activation(accT[:], trp[:], ACT.Copy)
                            agp = pp.tile([P, HID], F32, tag="agp")
                            nc.tensor.matmul(
                                out=agp[:], lhsT=accT[:], rhs=W1sb[:],
                                start=True, stop=True)
                            ab = bp.tile([P, HID], F16, tag="ab")
                            nc.scalar.activation(ab[:], agp[:], ACT.Copy)
                            nc.sync.dma_start(
                                agg1[j * P:(j + 1) * P, :], ab[:])
                            sq = bp.tile([P, HID], F16, tag="sq")
                            nc.vector.tensor_tensor(out=sq[:], in0=ab[:],
                                                    in1=ab[:], op=ALU.mult)
                            nc.tensor.matmul(
                                out=s0[:], lhsT=onesc[:], rhs=ab[:],
                                start=(j == 0), stop=(j == nblk - 1))
                            nc.tensor.matmul(
                                out=s1[:], lhsT=onesc[:], rhs=sq[:],
                                start=(j == 0), stop=(j == nblk - 1))
                    c0 = cp.tile([1, HID], F32, tag="c0l1")
                    nc.vector.tensor_copy(c0[:], s0[:])
                    c1 = cp.tile([1, HID], F32, tag="c1l1")
                    nc.vector.tensor_copy(c1[:], s1[:])
                    nc.sync.dma_start(st1_in[0:1, :], c0[:])
                    nc.sync.dma_start(st1_in[1:2, :], c1[:])

            def edge23(layer, table_t, wrow, numw, Hn, ad_own_t, agg_t,
                       st_in_ap, interleave):
                RH = numw + Hn
                with (
                    tc.tile_pool(name=f"g{layer}", bufs=2) as gp,
                    tc.tile_pool(name=f"w{layer}", bufs=2) as wp,
                    tc.tile_pool(name=f"b{layer}", bufs=3) as bp,
                    tc.tile_pool(name=f"pa{layer}", bufs=2, space="PSUM") as pa,
                    tc.tile_pool(name=f"pd{layer}", bufs=2, space="PSUM") as pd_,
                    tc.tile_pool(name=f"pq{layer}", bufs=1, space="PSUM") as pq,
                ):
                    s0 = pq.tile([1, numw], F32, tag="s0")
                    s1 = pq.tile([1, numw], F32, tag="s1")
                    for gd in lay.groups:
                        ts_g, te_g = gd["ts"], gd["te"]
                        ntg = te_g - ts_g
                        nbg = gd["j1"] - gd["j0"]
                        gs = gp.tile([P, ntg * wrow], F16, tag="gs")
                        for (q, ts, te) in gd["segs"]:
                            r0 = q * QSIZE
                            r1 = min(r0 + QSIZE, ROWS)
                            # SWDGE descriptor carveout is 1024/queue; keep
                            # each gather under it (7 tiles = 896 descs)
                            for u0 in range(ts, te, 7):
                                u1 = min(u0 + 7, te)
                                nc.gpsimd.dma_gather(
                                    out_ap=_ap3(gs, (u0 - ts_g) * wrow,
                                                [[wrow, u1 - u0], [1, wrow]]),
                                    in_ap=table_t[r0:r1, :],
                                    idxs_ap=islab[:, u0 * 8:u1 * 8],
                                    num_idxs=(u1 - u0) * P,
                                    num_idxs_reg=(u1 - u0) * P,
                                    elem_size=wrow)
                        selTc = gp.tile([P, ntg * P], F16, tag="selTc")
                        nc.sync.dma_start(
                            selTc[:], i_selT[:, ts_g * P:te_g * P])
                        sdbg = gp.tile([P, nbg * Hn], F16, tag="sdbg")
                        ad_ap = ad_own_t[:]
                        nc.sync.dma_start(
                            sdbg[:],
                            AP(ad_ap.tensor, ad_ap.offset + gd["j0"] * P * Hn,
                               [[Hn, P], [P * Hn, nbg], [1, Hn]]))
                        for j in range(gd["j0"], gd["j1"]):
                            K = int(lay.ktot[j])
                            selc = wp.tile([P, K * P], F16, tag="selc")
                            kb = 0
                            for (q, rs, re) in gd["runs"].get(j, []):
                                for t in range(rs, re):
                                    nc.vector.tensor_scalar(
                                        out=selc[:, (kb + t - rs) * P:
                                                 (kb + t - rs + 1) * P],
                                        in0=iota[:],
                                        scalar1=dcol[:, t:t + 1],
                                        scalar2=None, op0=ALU.is_equal)
                                kb += re - rs
                            sdps = pd_.tile([P, K * Hn], F32, tag="sdps")
                            ex = wp.tile([P, K * Hn], F16, tag="ex")
                            rhs = wp.tile([P, K * RH], F16, tag="rhs")
                            kb = 0
                            for (q, rs, re) in gd["runs"].get(j, []):
                                for t in range(rs, re):
                                    nc.tensor.matmul(
                                        out=sdps[:, (kb + t - rs) * Hn:
                                                 (kb + t - rs + 1) * Hn],
                                        lhsT=selTc[:, (t - ts_g) * P:
                                                   (t - ts_g + 1) * P],
                                        rhs=sdbg[:, (j - gd["j0"]) * Hn:
                                                 (j - gd["j0"] + 1) * Hn],
                                        start=True, stop=True)
                                kb += re - rs
                            kb = 0
                            for (q, rs, re) in gd["runs"].get(j, []):
                                nr = re - rs
                                l0 = rs - ts_g
                                lg = wp.tile([P, nr * Hn], F16, tag="lg")
                                nc.vector.tensor_tensor(
                                    out=lg[:],
                                    in0=_ap3(gs, l0 * wrow + numw,
                                             [[wrow, nr], [1, Hn]]),
                                    in1=sdps[:, kb * Hn:(kb + nr) * Hn],
                                    op=ALU.add)
                                lk = wp.tile([P, nr * Hn], F16, tag="lk")
                                nc.vector.scalar_tensor_tensor(
                                    out=lk[:], in0=lg[:], scalar=0.2,
                                    in1=lg[:], op0=ALU.mult, op1=ALU.max)
                                nc.scalar.activation(
                                    ex[:, kb * Hn:(kb + nr) * Hn], lk[:],
                                    ACT.Exp)
                                if interleave:
                                    ex_ap = _ap3(ex, kb * Hn,
                                                 [[Hn, nr], [0, D], [1, Hn]])
                                else:
                                    ex_ap = _ap3(ex, kb * Hn,
                                                 [[Hn, nr], [0, numw]])
                                nc.vector.tensor_tensor(
                                    out=_ap3(rhs, kb * RH,
                                             [[RH, nr], [1, numw]]),
                                    in0=_ap3(gs, l0 * wrow,
                                             [[wrow, nr], [1, numw]]),
                                    in1=ex_ap, op=ALU.mult)
                                nc.vector.tensor_copy(
                                    _ap3(rhs, kb * RH + numw,
                                         [[RH, nr], [1, Hn]]),
                                    ex[:, kb * Hn:(kb + nr) * Hn])
                                kb += nr
                            acc = pa.tile([P, RH], F32, tag="acc")
                            for kb in range(K):
                                nc.tensor.matmul(
                                    out=acc[:],
                                    lhsT=selc[:, kb * P:(kb + 1) * P],
                                    rhs=rhs[:, kb * RH:(kb + 1) * RH],
                                    start=(kb == 0), stop=(kb == K - 1))
                            den = bp.tile([P, Hn], F32, tag="den")
                            nc.vector.tensor_scalar_add(
                                den[:], acc[:, numw:numw + Hn], 1e-8)
                            rec = bp.tile([P, Hn], F32, tag="rec")
                            nc.vector.reciprocal(rec[:], den[:])
                            rec16 = bp.tile([P, Hn], F16, tag="rec16")
                            nc.vector.tensor_scalar(
                                out=rec16[:], in0=rec[:], scalar1=60000.0,
                                scalar2=None, op0=ALU.min)
                            accs = bp.tile([P, numw], F16, tag="accs")
                            nc.scalar.activation(accs[:], acc[:, 0:numw],
                                                 ACT.Copy)
                            ab = bp.tile([P, numw], F16, tag="ab")
                            if interleave:
                                rec_ap = _ap3(rec16, 0, [[0, D], [1, Hn]])
                            else:
                                rec_ap = _ap3(rec16, 0, [[1, Hn], [0, numw]])
                            nc.vector.tensor_tensor(
                                out=ab[:], in0=accs[:], in1=rec_ap,
                                op=ALU.mult)
                            nc.sync.dma_start(
                                agg_t[j * P:(j + 1) * P, :], ab[:])
                            sq = bp.tile([P, numw], F16, tag="sq")
                            nc.vector.tensor_tensor(out=sq[:], in0=ab[:],
                                                    in1=ab[:], op=ALU.mult)
                            nc.tensor.matmul(
                                out=s0[:], lhsT=onesc[:], rhs=ab[:],
                                start=(j == 0), stop=(j == nblk - 1))
                            nc.tensor.matmul(
                                out=s1[:], lhsT=onesc[:], rhs=sq[:],
                                start=(j == 0), stop=(j == nblk - 1))
                    c0 = cp.tile([1, numw], F32, tag=f"c0l{layer}")
                    nc.vector.tensor_copy(c0[:], s0[:])
                    c1 = cp.tile([1, numw], F32, tag=f"c1l{layer}")
                    nc.vector.tensor_copy(c1[:], s1[:])
                    nc.sync.dma_start(st_in_ap[0:1, :], c0[:])
                    nc.sync.dma_start(st_in_ap[1:2, :], c1[:])

            def stats_ar(st_in_ap, st_out_ap):
                nc.gpsimd.collective_compute(
                    "AllReduce", ALU.add,
                    ins=[st_in_ap], outs=[st_out_ap],
                    replica_groups=[list(range(NC_))])

            def bn_params(st_out_ap, CH, g_ap, b_ap, pool, tag=""):
                st0 = pool.tile([1, CH], F32, tag="st0" + tag)
                nc.sync.dma_start(st0[:], st_out_ap[0:1, :])
                st1 = pool.tile([1, CH], F32, tag="st1" + tag)
                nc.sync.dma_start(st1[:], st_out_ap[1:2, :])
                grow = pool.tile([1, CH], F32, tag="grow" + tag)
                nc.sync.dma_start(grow[:], g_ap[:])
                brow = pool.tile([1, CH], F32, tag="brow" + tag)
                nc.sync.dma_start(brow[:], b_ap[:])
                mu = pool.tile([1, CH], F32, tag="mu" + tag)
                nc.vector.tensor_scalar_mul(mu[:], st0[:], 1.0 / N)
                msq = pool.tile([1, CH], F32, tag="msq" + tag)
                nc.vector.tensor_scalar_mul(msq[:], st1[:], 1.0 / N)
                var = pool.tile([1, CH], F32, tag="var" + tag)
                nc.vector.tensor_tensor(out=var[:], in0=mu[:], in1=mu[:],
                                        op=ALU.mult)
                nc.vector.tensor_tensor(out=var[:], in0=msq[:], in1=var[:],
                                        op=ALU.subtract)
                nc.vector.tensor_scalar_add(var[:], var[:], EPS)
                sd = pool.tile([1, CH], F32, tag="sd" + tag)
                nc.scalar.activation(sd[:], var[:], ACT.Sqrt)
                rs_ = pool.tile([1, CH], F32, tag="rs" + tag)
                nc.vector.reciprocal(rs_[:], sd[:])
                arow = pool.tile([1, CH], F16, tag="arow" + tag)
                nc.vector.tensor_tensor(out=arow[:], in0=rs_[:], in1=grow[:],
                                        op=ALU.mult)
                brow2 = pool.tile([1, CH], F32, tag="brow2" + tag)
                nc.vector.tensor_tensor(out=brow2[:], in0=mu[:], in1=arow[:],
                                        op=ALU.mult)
                b2 = pool.tile([1, CH], F16, tag="b2" + tag)
                nc.vector.tensor_tensor(out=b2[:], in0=brow[:], in1=brow2[:],
                                        op=ALU.subtract)
                with tc.tile_pool(name=f"pbn{tag}", bufs=1,
                                  space="PSUM") as pb:
                    aps = pb.tile([P, CH], F32, tag="aps")
                    nc.tensor.matmul(out=aps[:], lhsT=ones1[:], rhs=arow[:],
                                     start=True, stop=True)
                    A = pool.tile([P, CH], F16, tag="Arep" + tag)
                    nc.scalar.activation(A[:], aps[:], ACT.Copy)
                    bps = pb.tile([P, CH], F32, tag="bps")
                    nc.tensor.matmul(out=bps[:], lhsT=ones1[:], rhs=b2[:],
                                     start=True, stop=True)
                    B = pool.tile([P, CH], F16, tag="Brep" + tag)
                    nc.scalar.activation(B[:], bps[:], ACT.Copy)
                return A, B

            def bn_elu(xsl, A, B, CH, pool, out_f32=False):
                z = pool.tile([P, CH], F16, tag="z")
                nc.vector.tensor_tensor(out=z[:], in0=xsl, in1=A[:],
                                        op=ALU.mult)
                nc.vector.tensor_tensor(out=z[:], in0=z[:], in1=B[:],
                                        op=ALU.add)
                m0 = pool.tile([P, CH], F16, tag="m0")
                nc.vector.tensor_scalar_min(m0[:], z[:], 0.0)
                e = pool.tile([P, CH], F16, tag="e")
                nc.scalar.activation(e[:], m0[:], ACT.Exp)
                p0 = pool.tile([P, CH], F16, tag="p0")
                nc.vector.tensor_scalar_max(p0[:], z[:], 0.0)
                act = pool.tile([P, CH], F32 if out_f32 else F16, tag="act")
                nc.vector.scalar_tensor_tensor(
                    out=act[:], in0=e[:], scalar=-1.0, in1=p0[:],
                    op0=ALU.add, op1=ALU.add)
                return act

            def build(layer, agg_t, st_out_ap, g_ap, b_ap, Wf_ap, outw, Hn,
                      wrow, t_own_t, t_full_t, ad_own_t):
                RWF = outw + 2 * Hn
                with (
                    tc.tile_pool(name=f"bs{layer}", bufs=3) as sp,
                    tc.tile_pool(name=f"bc{layer}", bufs=1) as cpool,
                    tc.tile_pool(name=f"bp{layer}", bufs=2, space="PSUM") as pp,
                    tc.tile_pool(name=f"bt{layer}", bufs=2, space="PSUM") as pt,
                ):
                    A, B = bn_params(st_out_ap, HID, g_ap, b_ap, cpool,
                                     f"b{layer}")
                    Wsb0 = cpool.tile([P, RWF], F16, tag="Wsb0")
                    nc.sync.dma_start(Wsb0[:], Wf_ap[0:P, :])
                    Wsb1 = cpool.tile([P, RWF], F16, tag="Wsb1")
                    nc.sync.dma_start(Wsb1[:], Wf_ap[P:2 * P, :])
                    Wsb = [Wsb0, Wsb1]
                    for j in range(nblk):
                        xb = sp.tile([P, HID], F16, tag="xb")
                        nc.sync.dma_start(
                            xb[:], agg_t[j * P:(j + 1) * P, :])
                        act = bn_elu(xb[:], A, B, HID, sp)
                        tp = pt.tile([P, RWF], F32, tag="tp")
                        for k in range(2):
                            trp = pp.tile([P, P], F16, tag="trp")
                            nc.tensor.transpose(
                                out=trp[:], in_=act[:, k * P:(k + 1) * P],
                                identity=ident[:])
                            atk = sp.tile([P, P], F16, tag="atk")
                            nc.scalar.activation(atk[:], trp[:], ACT.Copy)
                            nc.tensor.matmul(out=tp[:], lhsT=atk[:],
                                             rhs=Wsb[k][:],
                                             start=(k == 0), stop=(k == 1))
                        tt = sp.tile([P, wrow], F16, tag="tt")
                        nc.scalar.activation(tt[:, 0:outw + Hn],
                                             tp[:, 0:outw + Hn], ACT.Copy)
                        nc.vector.memset(tt[:, outw + Hn:wrow], 0.0)
                        nc.sync.dma_start(
                            t_own_t[j * P:(j + 1) * P, :], tt[:])
                        adr = sp.tile([P, Hn], F16, tag="adr")
                        nc.vector.tensor_copy(adr[:],
                                              tp[:, outw + Hn:outw + 2 * Hn])
                        nc.sync.dma_start(
                            ad_own_t[j * P:(j + 1) * P, :], adr[:])
                nc.gpsimd.collective_compute(
                    "AllGather", ALU.bypass,
                    ins=[t_own_t[:]], outs=[t_full_t[:]],
                    replica_groups=[list(range(NC_))])

            # ================= pipeline =================
            edge1()
            stats_ar(st1_in[:], st1_out[:])
            build(1, agg1[:], st1_out[:], i_g1[:], i_b1[:], i_W2f[:],
                  HID, H, W2ROW, t2_own[:], t2_full[:], ad2_own[:])
            edge23(2, t2_full, W2ROW, HID, H, ad2_own, agg2[:],
                   st2_in[:], True)
            stats_ar(st2_in[:], st2_out[:])
            build(2, agg2[:], st2_out[:], i_g2[:], i_b2[:], i_W3f[:],
                  D, 1, W3ROW, t3_own[:], t3_full[:], ad3_own[:])
            edge23(3, t3_full, W3ROW, D, 1, ad3_own, agg3[:],
                   st3_in[:], False)
            stats_ar(st3_in[:], st3_out[:])
            with (
                tc.tile_pool(name="f3", bufs=3) as sp,
                tc.tile_pool(name="f3c", bufs=1) as cpool,
            ):
                A3, B3 = bn_params(st3_out[:], D, i_g3[:], i_b3[:], cpool,
                                   "3")
                for j in range(nblk):
                    xb = sp.tile([P, D], F16, tag="xb")
                    nc.sync.dma_start(xb[:], agg3[j * P:(j + 1) * P, :])
                    act = bn_elu(xb[:], A3, B3, D, sp, out_f32=True)
                    nc.sync.dma_start(o_h3[j * P:(j + 1) * P, :], act[:])
    nc.compile()
    return nc


def kernel(**inputs):
    x = np.asarray(inputs["x"], np.float32)
    edge_index = np.asarray(inputs["edge_index"], np.int32)
    batch = np.asarray(inputs["batch"], np.int64)
    W1 = np.asarray(inputs["W1"], np.float32)
    a_src1 = np.asarray(inputs["a_src1"], np.float32)
    a_dst1 = np.asarray(inputs["a_dst1"], np.float32)
    g1 = np.asarray(inputs["g1"], np.float32); b1 = np.asarray(inputs["b1"], np.float32)
    W2 = np.asarray(inputs["W2"], np.float32)
    a_src2 = np.asarray(inputs["a_src2"], np.float32)
    a_dst2 = np.asarray(inputs["a_dst2"], np.float32)
    g2 = np.asarray(inputs["g2"], np.float32); b2 = np.asarray(inputs["b2"], np.float32)
    W3 = np.asarray(inputs["W3"], np.float32)
    a_src3 = np.asarray(inputs["a_src3"], np.float32)
    a_dst3 = np.asarray(inputs["a_dst3"], np.float32)
    g3 = np.asarray(inputs["g3"], np.float32); b3 = np.asarray(inputs["b3"], np.float32)
    Wc1 = np.asarray(inputs["Wc1"], np.float32); bc1 = np.asarray(inputs["bc1"], np.float32)
    gc1 = np.asarray(inputs["gc1"], np.float32); bnc1 = np.asarray(inputs["bnc1"], np.float32)
    Wc2 = np.asarray(inputs["Wc2"], np.float32); bc2 = np.asarray(inputs["bc2"], np.float32)
    Wc3 = np.asarray(inputs["Wc3"], np.float32); bc3 = np.asarray(inputs["bc3"], np.float32)

    nsh, nblk, nshp = _derived(N)
    perm = _perm()
    lay, xe_slab, selT_slab, dcol_slab, islab = _host_prep(
        x, edge_index, W1, a_src1, a_dst1)

    # --- weights, permuted to head-interleaved feature order
    W1blk = np.zeros((IN * H, HID), np.float32)
    for h in range(H):
        W1blk[h * IN:(h + 1) * IN, h * D:(h + 1) * D] = W1[:, h * D:(h + 1) * D]
    W1blk_p = W1blk[:, perm].astype(np.float16)

    av2, bv2 = _fold_attn(W2, a_src2, a_dst2, H, D)
    W2f = np.concatenate([W2[:, perm], av2, bv2], axis=1)  # cols: pi-h, asrc, adst
    W2f_p = W2f[perm, :].astype(np.float16)                # rows: pi (act1 order)

    av3 = (W3 @ a_src3[0])[:, None]
    bv3 = (W3 @ a_dst3[0])[:, None]
    W3f = np.concatenate([W3, av3, bv3], axis=1)
    W3f_p = W3f[perm, :].astype(np.float16)

    nc = _build_program(lay, nblk, nshp)

    consts = {
        "i_ident": np.eye(P, dtype=np.float16),
        "i_iota": np.tile(np.arange(P, dtype=np.float16)[None, :], (P, 1)),
        "i_ones1": np.ones((1, P), np.float16),
        "i_W1": W1blk_p, "i_W2f": W2f_p, "i_W3f": W3f_p,
        "i_g1": g1[perm][None, :], "i_b1": b1[perm][None, :],
        "i_g2": g2[perm][None, :], "i_b2": b2[perm][None, :],
        "i_g3": g3[None, :], "i_b3": b3[None, :],
    }
    in_maps = []
    for c in range(NC_):
        in_maps.append({
            "i_xe": xe_slab[c], "i_dcol": dcol_slab[c],
            "i_selT": selT_slab[c], "i_islab": islab[c], **consts})

    res = bass_utils.run_bass_kernel_spmd(nc, in_maps,
                                          core_ids=list(range(NC_)))
    h3 = np.concatenate(
        [res.results[c]["o_h3"][:nsh] for c in range(NC_)], axis=0)

    # ---- host: graph pooling + classifier MLP (fp32 mirror of reference)
    counts = np.bincount(batch, minlength=G).astype(np.float32)
    h_sum = np.zeros((G, D), np.float32)
    np.add.at(h_sum, batch, h3)
    h_mean = h_sum / (counts[:, None] + 1.0)
    h_max = np.full((G, D), -np.inf, np.float32)
    np.maximum.at(h_max, batch, h3)
    h_max = np.maximum(h_max, 0.0)
    gfeat = np.concatenate([h_mean, h_max], axis=1)
    z = gfeat @ Wc1 + bc1
    mu = z.mean(0); var = z.var(0)
    z = (z - mu) / np.sqrt(var + EPS) * gc1 + bnc1
    z = np.maximum(z, 0.0)
    z = np.maximum(z @ Wc2 + bc2, 0.0)
    return (z @ Wc3 + bc3).astype(np.float32)
